# revision 10
# baseline (speedup 1.0000x reference)
"""Trainium2 Bass kernel for nn_Couple_loss_62380105007762.

Loss = w0 * MSE + w1 * KLD + w2 * CE where
  sig(x)  = 2 * x[:, 0].sum(axis=F)                      (inverse SSQ-STFT, real channel only)
  MSE     = sum((sig(output_rec) - sig(target_rec))**2)
  KLD     = -0.5 * sum(1 + log_var - mean**2 - exp(log_var))
  CE      = mean cross-entropy(output_clas, target_clas)

Sharding: data-parallel over the batch dim (64 rows -> 8 cores x 8 rows).
Each core emits per-shard partial sums [8, 4] (sq_h1, sq_h2, kld, ce rows);
host psums the shards and applies the 3 loss weights.

v4 (v1 72.0us, v2 38.6us, v3 31.7us):
  - fp8e4 rec data (4x traffic; ~9e-4 rel err), interleaved [F, (b, {o,t}, T)].
  - DMA under an f32 VIEW of the fp8 tiles: the DGE splits descriptors at
    2048 ELEMENTS, so fp8 APs cap packets at 2 KB (~160 GB/s/queue); the
    same bytes as [128, 1024] f32 give 4 KB packets.
  - DMA triggers hoisted into the `main` block right after the runtime
    preamble call, ahead of the framework start barrier (~1.5us earlier;
    they have no data dependencies).
  - three queues: sync [b0,b2,b4], scalar [b1,b3,b5], gpsimd [smalls,b6,b7]
    (SWDGE has ~4.5us first-packet latency -> give it the last-needed
    chunks); per-HWDGE-queue rate caps at ~160-200 GB/s, so 2 queues
    cannot reach the ~360-400 GB/s HBM line rate but 3 can.
  - DoubleRow fp8 matmuls (32 total): one matmul = sum_f(o) - sum_f(t).
  - two [8, 2048] psum halves; ONE [8, 2048] square+accum per half (Tile
    tracks deps at tile granularity, so per-bank squares all waited for
    the last matmul anyway; one big ACT op has the least tail).
  - no device-side weighted combine: the [8, 4] partials DMA out right
    after the last accumulator read.
"""

import numpy as np
import ml_dtypes
from contextlib import ExitStack

import concourse.bass as bass
import concourse.tile as tile
from concourse import mybir
from concourse.bass_utils import run_bass_kernel_spmd

N_CORES = 8
B, Z, F, T, C = 64, 256, 128, 2048, 5
BS = B // N_CORES   # batch rows per core
HB = BS // 2        # rows per psum half
WCOL = BS * 2 * T   # interleaved free dim: 32768 fp8 columns
WCOL32 = WCOL // 4  # same bytes as f32 columns
N_CHUNK = 512       # matmul output free dim (PSUM bank limit in fp32)
KQ = T // N_CHUNK   # 4 output slices per b
N_WARM = 7          # dummy matmuls bridging PE barrier-exit -> first data

FP8 = mybir.dt.float8e4
NP_FP8 = ml_dtypes.float8_e4m3
FP32 = mybir.dt.float32
AX = mybir.AxisListType
ALU = mybir.AluOpType
ACTF = mybir.ActivationFunctionType
DR = mybir.MatmulPerfMode.DoubleRow
ET = mybir.EngineType

# packed smalls layout: [BS, SM_W] f32
SM_MEAN = 0               # cols [0, 256)    mean
SM_LV = Z                 # cols [256, 512)  log_var
SM_OC = 2 * Z             # cols [512, 517)  output_clas
SM_OH = 2 * Z + C         # cols [517, 522)  one-hot(target_clas)
SM_W = 2 * Z + 2 * C

# out columns
NO = 4                    # [sq_h1, sq_h2, kld, ce]


def build_bass(legalize: bool = True):
    nc = bass.Bass()

    ot_rec = nc.declare_dram_parameter("ot_rec", [F, WCOL32], FP32, isOutput=False)
    smalls = nc.declare_dram_parameter("smalls", [BS, SM_W], FP32, isOutput=False)
    out = nc.declare_dram_parameter("out", [BS, NO], FP32, isOutput=True)

    hoist_hwdge = []   # instruction names to move right after the main InstCall
    hoist_pool = []    # ... and after the Pool preamble memsets

    with tile.TileContext(nc) as tc:
        with ExitStack() as ctx:
            const_pool = ctx.enter_context(tc.tile_pool(name="const", bufs=1))
            d_pool = ctx.enter_context(tc.tile_pool(name="dpool", bufs=BS))
            ps_pool = ctx.enter_context(tc.tile_pool(name="ps", bufs=1, space="PSUM"))
            small = ctx.enter_context(tc.tile_pool(name="small", bufs=1))

            # ---- big data chunks; DMA issued under an f32 view ----
            # chunk b = [128, 4096] fp8 = o_b | t_b.
            QUEUES = {0: nc.sync, 1: nc.scalar, 2: nc.sync, 3: nc.scalar,
                      4: nc.sync, 5: nc.scalar, 6: nc.gpsimd, 7: nc.gpsimd}
            sm_t = small.tile([BS, SM_W], FP32, tag="sm")
            i_sm = nc.gpsimd.dma_start(sm_t[:], smalls[:, :])
            hoist_pool.append(i_sm.ins.name)
            chunks = []
            for b in range(BS):
                ch = d_pool.tile([F, 2 * T], FP8, tag="d")
                sl32 = slice(b * 2 * T // 4, (b + 1) * 2 * T // 4)
                i_d = QUEUES[b].dma_start(ch[:].bitcast(FP32), ot_rec[:, sl32])
                (hoist_pool if b >= 6 else hoist_hwdge).append(i_d.ins.name)
                chunks.append(ch)

            # ---- constants (no DMA): selector weights + warmup junk ----
            # W[:, 8] = +1, W[:, 24] = -1, rest 0.  DoubleRow stationary for
            # batch row b: W viewed as [128, j:2(x16), m:8] at offset 8-b
            # => (j=0, m=b) hits col 8 (+1), (j=1, m=b) hits col 24 (-1).
            w_sel = const_pool.tile([F, 32], FP8, tag="wsel")
            i1 = nc.vector.memset(w_sel[:], 0.0)
            i2 = nc.vector.memset(w_sel[:, 8:9], 1.0)
            i3 = nc.vector.memset(w_sel[:, 24:25], -1.0)
            warm_in = const_pool.tile([F, N_CHUNK], FP8, tag="warmin")
            i4 = nc.vector.memset(warm_in[:], 0.0)
            hoist_hwdge += [i1.ins.name, i2.ins.name, i3.ins.name, i4.ins.name]

            # psum: two [8, 2048] halves = all 8 banks
            ps_h1 = ps_pool.tile([BS, T], FP32, tag="h1")
            ps_h2 = ps_pool.tile([BS, T], FP32, tag="h2")
            ps_h = [ps_h1, ps_h2]
            # out tile: [8, 4] = sq_h1 | sq_h2 | kld | ce
            sums = small.tile([BS, NO], FP32, tag="sums")

            # ---- PE warmup: HAM unthrottles after ~3.4us of activity.
            # Writes [1, 512] garbage into half2 bank0; the real b4 matmul
            # opens its accumulation group with start=True, clearing it.
            for i in range(N_WARM):
                nc.tensor.matmul(ps_h2[0:1, 0:N_CHUNK], w_sel[:, 0:1],
                                 warm_in[:], start=True, stop=True,
                                 skip_group_check=True)

            # ---- KLD / CE on the packed smalls tile (off critical path) ----
            m_t = sm_t[:, SM_MEAN:SM_MEAN + Z]
            lv_t = sm_t[:, SM_LV:SM_LV + Z]
            oc_t = sm_t[:, SM_OC:SM_OC + C]
            oh_t = sm_t[:, SM_OH:SM_OH + C]

            msq_sum = small.tile([BS, 1], FP32, tag="msq")
            e_sum = small.tile([BS, 1], FP32, tag="esum")
            lv_sum = small.tile([BS, 1], FP32, tag="lvsum")
            kl_junk = small.tile([BS, Z], FP32, tag="klj")
            kl_junk2 = small.tile([BS, Z], FP32, tag="klj2")
            kl_tmp = small.tile([BS, 1], FP32, tag="kltmp")
            nc.vector.tensor_tensor(kl_junk[:], m_t, m_t, ALU.mult)
            nc.vector.reduce_sum(msq_sum[:], kl_junk[:], axis=AX.X)
            nc.scalar.activation(kl_junk2[:], lv_t, ACTF.Exp, accum_out=e_sum[:])
            nc.vector.reduce_sum(lv_sum[:], lv_t, axis=AX.X)
            nc.vector.tensor_tensor(kl_tmp[:], lv_sum[:], msq_sum[:], ALU.subtract)
            nc.vector.tensor_tensor(sums[:, 2:3], kl_tmp[:], e_sum[:], ALU.subtract)

            # CE rows: ce_row = rowmax + log(sum(exp(oc - rowmax))) - oc[b, y_b]
            rowmax = small.tile([BS, 1], FP32, tag="rmax")
            nmax = small.tile([BS, 1], FP32, tag="nmax")
            sumexp = small.tile([BS, 1], FP32, tag="sexp")
            lse = small.tile([BS, 1], FP32, tag="lse")
            picked = small.tile([BS, 1], FP32, tag="picked")
            ce_junk = small.tile([BS, C], FP32, tag="cej")
            ce_junk2 = small.tile([BS, C], FP32, tag="cej2")
            ce_tmp = small.tile([BS, 1], FP32, tag="cetmp")
            nc.vector.reduce_max(rowmax[:], oc_t, axis=AX.X)
            nc.vector.tensor_scalar_mul(nmax[:], rowmax[:], -1.0)
            nc.scalar.activation(
                ce_junk[:], oc_t, ACTF.Exp, bias=nmax[:], accum_out=sumexp[:]
            )
            nc.scalar.activation(lse[:], sumexp[:], ACTF.Ln)
            nc.vector.tensor_tensor(ce_junk2[:], oc_t, oh_t, ALU.mult)
            nc.vector.reduce_sum(picked[:], ce_junk2[:], axis=AX.X)
            nc.vector.tensor_tensor(ce_tmp[:], rowmax[:], lse[:], ALU.add)
            nc.vector.tensor_tensor(sums[:, 3:4], ce_tmp[:], picked[:],
                                    ALU.subtract)

            # ---- main MSE stream ----
            # DoubleRow: out[m, n] = sum_f W3[f, 0, m]*ch3[f, 0, n]
            #                      + sum_f W3[f, 1, m]*ch3[f, 1, n]
            #          = sum_f o[b, f, n] - sum_f t[b, f, n]  for m == b%...
            # (b0-3 land in rows 0-3 of half1; b4-7 in rows 4-7 of half2;
            # the unused rows stay 0 and add nothing to the accumulated sum)
            w3 = w_sel[:].rearrange("p (j m) -> p j m", j=2)  # [128, 2, 16]
            for b in range(BS):
                wb = w3[:, :, 8 - b:16 - b]                    # [128, 2, 8]
                c3 = chunks[b][:].rearrange("p (j n) -> p j n", j=2)
                h = ps_h[b // HB]
                for k in range(KQ):
                    nc.tensor.matmul(
                        h[:, k * N_CHUNK:(k + 1) * N_CHUNK],
                        wb, c3[:, :, k * N_CHUNK:(k + 1) * N_CHUNK],
                        start=(b % HB == 0),
                        stop=(b % HB == HB - 1),
                        perf_mode=DR,
                    )
                if b % HB == HB - 1:
                    half = b // HB
                    sq_junk = small.tile([BS, T], FP32, tag=f"sqj{half}")
                    nc.scalar.activation(
                        sq_junk[:], ps_h[half][:], ACTF.Square,
                        accum_out=sums[:, half:half + 1],
                    )

            nc.sync.dma_start(out[:, :], sums[:])

    if legalize:
        _legalize_multi_waits(nc)
    _hoist_preamble(nc, hoist_hwdge, hoist_pool)
    mybir.codegen_inst_isa_subclasses(nc)
    return nc


def _hoist_preamble(nc, names_after_call, names_pool):
    """Move dependency-free DMA triggers / memsets from the body block into
    `main`, ahead of the framework start barrier: HWDGE triggers + DVE
    memsets right after the runtime-preamble InstCall, SWDGE (Pool)
    triggers after the Pool preamble memsets (DGE ring init)."""
    fn = nc.m.functions[0]
    main = fn.blocks[0]
    assert main.name == "main"
    wanted = set(names_after_call) | set(names_pool)
    moved = {}
    for blk in fn.blocks[1:]:
        keep = []
        for inst in blk.instructions:
            if inst.name in wanted:
                moved[inst.name] = inst
            else:
                keep.append(inst)
        blk.instructions = keep

    new_main = []
    for inst in main.instructions:
        new_main.append(inst)
        if type(inst).__name__ == "InstCall":
            for n in names_after_call:
                if n in moved:
                    new_main.append(moved[n])
    final = []
    pool_done = False
    for inst in new_main:
        if (not pool_done and type(inst).__name__ == "InstDrain"
                and inst.engine == ET.Pool):
            for n in names_pool:
                if n in moved:
                    final.append(moved[n])
            pool_done = True
        final.append(inst)
    main.instructions = final


def _legalize_multi_waits(nc):
    """walrus rejects TPB compute instructions carrying more than one sync
    wait. Hoist extra waits onto standalone InstEventSemaphore instructions
    on the same engine. DMA instructions keep their waits (DGE path).
    """
    for fn in nc.m.functions:
        for blk in fn.blocks:
            new_insts = []
            for inst in blk.instructions:
                si = inst.sync_info
                tname = type(inst).__name__
                if (
                    si is not None
                    and si.on_wait
                    and len(si.on_wait) > 1
                    and tname != "InstEventSemaphore"
                ):
                    for i, w in enumerate(si.on_wait):
                        new_insts.append(
                            mybir.InstEventSemaphore(
                                name=f"{inst.name}_hoistw{i}",
                                engine=inst.engine,
                                ins=[],
                                outs=[],
                                sync_info=mybir.SyncInfo(on_wait=[w], on_update=[]),
                            )
                        )
                    inst.sync_info = mybir.SyncInfo(
                        on_wait=[], on_update=si.on_update
                    )
                new_insts.append(inst)
            blk.instructions = new_insts


_NC_CACHE = {}


def _get_nc():
    if "nc" not in _NC_CACHE:
        _NC_CACHE["nc"] = build_bass()
    return _NC_CACHE["nc"]


def make_in_maps(inputs) -> list[dict]:
    o = np.asarray(inputs["output_rec"], dtype=np.float32)
    t = np.asarray(inputs["target_rec"], dtype=np.float32)
    mean = np.asarray(inputs["mean"], dtype=np.float32)
    log_var = np.asarray(inputs["log_var"], dtype=np.float32)
    oclas = np.asarray(inputs["output_clas"], dtype=np.float32)
    tclas = np.asarray(inputs["target_clas"]).astype(np.int64)

    # Only the real channel contributes to the inverse SSQ-STFT. Quantize
    # to fp8e4 (measured ~9e-4 rel err on the loss; tolerance is 2e-2).
    o_q = o[:, 0].astype(NP_FP8)  # [B, F, T]
    t_q = t[:, 0].astype(NP_FP8)

    onehot = np.zeros((B, C), dtype=np.float32)
    onehot[np.arange(B), tclas] = 1.0

    in_maps = []
    for c in range(N_CORES):
        s = slice(c * BS, (c + 1) * BS)
        # [BS, F, T] x2 -> [F, BS, {o,t}, T] -> f32 view [F, 8192]
        ot = np.empty((F, BS, 2, T), dtype=NP_FP8)
        ot[:, :, 0, :] = o_q[s].transpose(1, 0, 2)
        ot[:, :, 1, :] = t_q[s].transpose(1, 0, 2)
        sm = np.zeros((BS, SM_W), dtype=np.float32)
        sm[:, SM_MEAN:SM_MEAN + Z] = mean[s]
        sm[:, SM_LV:SM_LV + Z] = log_var[s]
        sm[:, SM_OC:SM_OC + C] = oclas[s]
        sm[:, SM_OH:SM_OH + C] = onehot[s]
        in_maps.append(
            {"ot_rec": ot.reshape(F, WCOL).view(np.float32), "smalls": sm}
        )
    return in_maps


def kernel(**inputs) -> np.ndarray:
    in_maps = make_in_maps(inputs)
    nc = _get_nc()
    res = run_bass_kernel_spmd(nc, in_maps, list(range(N_CORES)))
    w = np.asarray(inputs["weight"], dtype=np.float64)
    total = 0.0
    for r in res.results:
        p = np.asarray(r["out"], dtype=np.float64)  # [8, 4]
        sq = p[:, 0].sum() + p[:, 1].sum()
        kld = p[:, 2].sum()
        ce = p[:, 3].sum()
        total += (4.0 * w[0] * sq
                  - 0.5 * w[1] * (kld + BS * Z)
                  + (w[2] / B) * ce)
    return np.float32(total)


# revision 11
# speedup vs baseline: 1.3313x; 1.3313x over previous
"""Trainium2 Bass kernel for nn_Couple_loss_62380105007762.

Loss = w0 * MSE + w1 * KLD + w2 * CE where
  sig(x)  = 2 * x[:, 0].sum(axis=F)                      (inverse SSQ-STFT, real channel only)
  MSE     = sum((sig(output_rec) - sig(target_rec))**2)
  KLD     = -0.5 * sum(1 + log_var - mean**2 - exp(log_var))
  CE      = mean cross-entropy(output_clas, target_clas)

Sharding: data-parallel over the batch dim (64 rows -> 8 cores x 8 rows).
Each core emits per-shard partial sums [8, 4] (sq_h1, sq_h2, kld, ce rows);
host psums the shards and applies the 3 loss weights.

v4 (v1 72.0us, v2 38.6us, v3 31.7us):
  - fp8e4 rec data (4x traffic; ~9e-4 rel err), interleaved [F, (b, {o,t}, T)].
  - DMA under an f32 VIEW of the fp8 tiles: the DGE splits descriptors at
    2048 ELEMENTS, so fp8 APs cap packets at 2 KB (~160 GB/s/queue); the
    same bytes as [128, 1024] f32 give 4 KB packets.
  - DMA triggers hoisted into the `main` block right after the runtime
    preamble call, ahead of the framework start barrier (~1.5us earlier;
    they have no data dependencies).
  - three queues: sync [b0,b2,b4], scalar [b1,b3,b5], gpsimd [smalls,b6,b7]
    (SWDGE has ~4.5us first-packet latency -> give it the last-needed
    chunks); per-HWDGE-queue rate caps at ~160-200 GB/s, so 2 queues
    cannot reach the ~360-400 GB/s HBM line rate but 3 can.
  - DoubleRow fp8 matmuls (32 total): one matmul = sum_f(o) - sum_f(t).
  - two [8, 2048] psum halves; ONE [8, 2048] square+accum per half (Tile
    tracks deps at tile granularity, so per-bank squares all waited for
    the last matmul anyway; one big ACT op has the least tail).
  - no device-side weighted combine: the [8, 4] partials DMA out right
    after the last accumulator read.
"""

import numpy as np
import ml_dtypes
from contextlib import ExitStack

import concourse.bass as bass
import concourse.tile as tile
from concourse import mybir
from concourse.bass_utils import run_bass_kernel_spmd

N_CORES = 8
B, Z, F, T, C = 64, 256, 128, 2048, 5
BS = B // N_CORES   # batch rows per core
HB = BS // 2        # rows per psum half
WCOL = BS * 2 * T   # interleaved free dim: 32768 fp8 columns
WCOL32 = WCOL // 4  # same bytes as f32 columns
N_CHUNK = 512       # matmul output free dim (PSUM bank limit in fp32)
KQ = T // N_CHUNK   # 4 output slices per b
N_WARM = 12         # dummy matmuls bridging PE barrier-exit -> first 1MB chunk

FP8 = mybir.dt.float8e4
NP_FP8 = ml_dtypes.float8_e4m3
FP32 = mybir.dt.float32
AX = mybir.AxisListType
ALU = mybir.AluOpType
ACTF = mybir.ActivationFunctionType
DR = mybir.MatmulPerfMode.DoubleRow
ET = mybir.EngineType

# packed smalls layout: [BS, SM_W] f32
SM_MEAN = 0               # cols [0, 256)    mean
SM_LV = Z                 # cols [256, 512)  log_var
SM_OC = 2 * Z             # cols [512, 517)  output_clas
SM_OH = 2 * Z + C         # cols [517, 522)  one-hot(target_clas)
SM_W = 2 * Z + 2 * C

# out columns
NO = 4                    # [sq_h1, sq_h2, kld, ce]


def build_bass(legalize: bool = True):
    nc = bass.Bass()

    ot_rec = nc.declare_dram_parameter("ot_rec", [F, WCOL32], FP32, isOutput=False)
    smalls = nc.declare_dram_parameter("smalls", [BS, SM_W], FP32, isOutput=False)
    out = nc.declare_dram_parameter("out", [BS, NO], FP32, isOutput=True)

    with tile.TileContext(nc) as tc:
        with ExitStack() as ctx:
            const_pool = ctx.enter_context(tc.tile_pool(name="const", bufs=1))
            d_pool = ctx.enter_context(tc.tile_pool(name="dpool", bufs=BS // 2))
            ps_pool = ctx.enter_context(tc.tile_pool(name="ps", bufs=1, space="PSUM"))
            small = ctx.enter_context(tc.tile_pool(name="small", bufs=1))

            # ---- big data chunks; DMA issued under an f32 view ----
            # chunk b = [128, 4096] fp8 = o_b | t_b.
            QUEUES = {0: nc.sync, 1: nc.scalar, 2: nc.sync, 3: nc.scalar}
            sm_t = small.tile([BS, SM_W], FP32, tag="sm")
            nc.gpsimd.dma_start(sm_t[:], smalls[:, :])
            # 2-b chunks: [128, 8192] fp8 = 1 MB, 8 KB f32-view descriptors
            # (HWDGE queue rate scales with descriptor size: ~205 GB/s at
            # 8 KB vs ~150-165 below; descs split at 2048 elements)
            pairs = []
            for p in range(BS // 2):
                ch = d_pool.tile([F, 4 * T], FP8, tag="d")
                sl32 = slice(p * T, (p + 1) * T)
                QUEUES[p].dma_start(ch[:].bitcast(FP32), ot_rec[:, sl32])
                pairs.append(ch)
            chunks = []
            for b in range(BS):
                ch = pairs[b // 2]
                half = (b % 2) * 2 * T
                chunks.append(ch[:, half:half + 2 * T])

            # ---- constants (no DMA): selector weights + warmup junk ----
            # W[:, 8] = +1, W[:, 24] = -1, rest 0.  DoubleRow stationary for
            # batch row b: W viewed as [128, j:2(x16), m:8] at offset 8-b
            # => (j=0, m=b) hits col 8 (+1), (j=1, m=b) hits col 24 (-1).
            w_sel = const_pool.tile([F, 32], FP8, tag="wsel")
            nc.vector.memset(w_sel[:], 0.0)
            nc.vector.memset(w_sel[:, 8:9], 1.0)
            nc.vector.memset(w_sel[:, 24:25], -1.0)
            warm_in = const_pool.tile([F, N_CHUNK], FP8, tag="warmin")
            nc.vector.memset(warm_in[:], 0.0)

            # psum: two [8, 2048] halves = all 8 banks
            ps_h1 = ps_pool.tile([BS, T], FP32, tag="h1")
            ps_h2 = ps_pool.tile([BS, T], FP32, tag="h2")
            ps_h = [ps_h1, ps_h2]
            # out tile: [8, 4] = sq_h1 | sq_h2 | kld | ce
            sums = small.tile([BS, NO], FP32, tag="sums")

            # ---- PE warmup: HAM unthrottles after ~3.4us of activity.
            # Writes [1, 512] garbage into half2 bank0; the real b4 matmul
            # opens its accumulation group with start=True, clearing it.
            for i in range(N_WARM):
                nc.tensor.matmul(ps_h2[0:1, 0:N_CHUNK], w_sel[:, 0:1],
                                 warm_in[:], start=True, stop=True,
                                 skip_group_check=True)

            # ---- KLD / CE on the packed smalls tile (off critical path) ----
            m_t = sm_t[:, SM_MEAN:SM_MEAN + Z]
            lv_t = sm_t[:, SM_LV:SM_LV + Z]
            oc_t = sm_t[:, SM_OC:SM_OC + C]
            oh_t = sm_t[:, SM_OH:SM_OH + C]

            msq_sum = small.tile([BS, 1], FP32, tag="msq")
            e_sum = small.tile([BS, 1], FP32, tag="esum")
            lv_sum = small.tile([BS, 1], FP32, tag="lvsum")
            kl_junk = small.tile([BS, Z], FP32, tag="klj")
            kl_junk2 = small.tile([BS, Z], FP32, tag="klj2")
            kl_tmp = small.tile([BS, 1], FP32, tag="kltmp")
            nc.vector.tensor_tensor(kl_junk[:], m_t, m_t, ALU.mult)
            nc.vector.reduce_sum(msq_sum[:], kl_junk[:], axis=AX.X)
            nc.scalar.activation(kl_junk2[:], lv_t, ACTF.Exp, accum_out=e_sum[:])
            nc.vector.reduce_sum(lv_sum[:], lv_t, axis=AX.X)
            nc.vector.tensor_tensor(kl_tmp[:], lv_sum[:], msq_sum[:], ALU.subtract)
            nc.vector.tensor_tensor(sums[:, 2:3], kl_tmp[:], e_sum[:], ALU.subtract)

            # CE rows: ce_row = rowmax + log(sum(exp(oc - rowmax))) - oc[b, y_b]
            rowmax = small.tile([BS, 1], FP32, tag="rmax")
            nmax = small.tile([BS, 1], FP32, tag="nmax")
            sumexp = small.tile([BS, 1], FP32, tag="sexp")
            lse = small.tile([BS, 1], FP32, tag="lse")
            picked = small.tile([BS, 1], FP32, tag="picked")
            ce_junk = small.tile([BS, C], FP32, tag="cej")
            ce_junk2 = small.tile([BS, C], FP32, tag="cej2")
            ce_tmp = small.tile([BS, 1], FP32, tag="cetmp")
            nc.vector.reduce_max(rowmax[:], oc_t, axis=AX.X)
            nc.vector.tensor_scalar_mul(nmax[:], rowmax[:], -1.0)
            nc.scalar.activation(
                ce_junk[:], oc_t, ACTF.Exp, bias=nmax[:], accum_out=sumexp[:]
            )
            nc.scalar.activation(lse[:], sumexp[:], ACTF.Ln)
            nc.vector.tensor_tensor(ce_junk2[:], oc_t, oh_t, ALU.mult)
            nc.vector.reduce_sum(picked[:], ce_junk2[:], axis=AX.X)
            nc.vector.tensor_tensor(ce_tmp[:], rowmax[:], lse[:], ALU.add)
            nc.vector.tensor_tensor(sums[:, 3:4], ce_tmp[:], picked[:],
                                    ALU.subtract)

            # ---- main MSE stream ----
            # DoubleRow: out[m, n] = sum_f W3[f, 0, m]*ch3[f, 0, n]
            #                      + sum_f W3[f, 1, m]*ch3[f, 1, n]
            #          = sum_f o[b, f, n] - sum_f t[b, f, n]  for m == b%...
            # (b0-3 land in rows 0-3 of half1; b4-7 in rows 4-7 of half2;
            # the unused rows stay 0 and add nothing to the accumulated sum)
            w3 = w_sel[:].rearrange("p (j m) -> p j m", j=2)  # [128, 2, 16]
            for b in range(BS):
                wb = w3[:, :, 8 - b:16 - b]                    # [128, 2, 8]
                c3 = chunks[b].rearrange("p (j n) -> p j n", j=2)
                h = ps_h[b // HB]
                for k in range(KQ):
                    nc.tensor.matmul(
                        h[:, k * N_CHUNK:(k + 1) * N_CHUNK],
                        wb, c3[:, :, k * N_CHUNK:(k + 1) * N_CHUNK],
                        start=(b % HB == 0),
                        stop=(b % HB == HB - 1),
                        perf_mode=DR,
                    )
                if b % HB == HB - 1:
                    half = b // HB
                    sq_junk = small.tile([BS, T], FP32, tag=f"sqj{half}")
                    nc.scalar.activation(
                        sq_junk[:], ps_h[half][:], ACTF.Square,
                        accum_out=sums[:, half:half + 1],
                    )

            nc.sync.dma_start(out[:, :], sums[:])

    if legalize:
        _legalize_multi_waits(nc)
    mybir.codegen_inst_isa_subclasses(nc)
    return nc


def _legalize_multi_waits(nc):
    """walrus rejects TPB compute instructions carrying more than one sync
    wait. Hoist extra waits onto standalone InstEventSemaphore instructions
    on the same engine. DMA instructions keep their waits (DGE path).
    """
    for fn in nc.m.functions:
        for blk in fn.blocks:
            new_insts = []
            for inst in blk.instructions:
                si = inst.sync_info
                tname = type(inst).__name__
                if (
                    si is not None
                    and si.on_wait
                    and len(si.on_wait) > 1
                    and tname != "InstEventSemaphore"
                ):
                    for i, w in enumerate(si.on_wait):
                        new_insts.append(
                            mybir.InstEventSemaphore(
                                name=f"{inst.name}_hoistw{i}",
                                engine=inst.engine,
                                ins=[],
                                outs=[],
                                sync_info=mybir.SyncInfo(on_wait=[w], on_update=[]),
                            )
                        )
                    inst.sync_info = mybir.SyncInfo(
                        on_wait=[], on_update=si.on_update
                    )
                new_insts.append(inst)
            blk.instructions = new_insts


_NC_CACHE = {}


def _get_nc():
    if "nc" not in _NC_CACHE:
        _NC_CACHE["nc"] = build_bass()
    return _NC_CACHE["nc"]


def make_in_maps(inputs) -> list[dict]:
    o = np.asarray(inputs["output_rec"], dtype=np.float32)
    t = np.asarray(inputs["target_rec"], dtype=np.float32)
    mean = np.asarray(inputs["mean"], dtype=np.float32)
    log_var = np.asarray(inputs["log_var"], dtype=np.float32)
    oclas = np.asarray(inputs["output_clas"], dtype=np.float32)
    tclas = np.asarray(inputs["target_clas"]).astype(np.int64)

    # Only the real channel contributes to the inverse SSQ-STFT. Quantize
    # to fp8e4 (measured ~9e-4 rel err on the loss; tolerance is 2e-2).
    o_q = o[:, 0].astype(NP_FP8)  # [B, F, T]
    t_q = t[:, 0].astype(NP_FP8)

    onehot = np.zeros((B, C), dtype=np.float32)
    onehot[np.arange(B), tclas] = 1.0

    in_maps = []
    for c in range(N_CORES):
        s = slice(c * BS, (c + 1) * BS)
        # [BS, F, T] x2 -> [F, BS, {o,t}, T] -> f32 view [F, 8192]
        ot = np.empty((F, BS, 2, T), dtype=NP_FP8)
        ot[:, :, 0, :] = o_q[s].transpose(1, 0, 2)
        ot[:, :, 1, :] = t_q[s].transpose(1, 0, 2)
        sm = np.zeros((BS, SM_W), dtype=np.float32)
        sm[:, SM_MEAN:SM_MEAN + Z] = mean[s]
        sm[:, SM_LV:SM_LV + Z] = log_var[s]
        sm[:, SM_OC:SM_OC + C] = oclas[s]
        sm[:, SM_OH:SM_OH + C] = onehot[s]
        in_maps.append(
            {"ot_rec": ot.reshape(F, WCOL).view(np.float32), "smalls": sm}
        )
    return in_maps


def kernel(**inputs) -> np.ndarray:
    in_maps = make_in_maps(inputs)
    nc = _get_nc()
    res = run_bass_kernel_spmd(nc, in_maps, list(range(N_CORES)))
    w = np.asarray(inputs["weight"], dtype=np.float64)
    total = 0.0
    for r in res.results:
        p = np.asarray(r["out"], dtype=np.float64)  # [8, 4]
        sq = p[:, 0].sum() + p[:, 1].sum()
        kld = p[:, 2].sum()
        ce = p[:, 3].sum()
        total += (4.0 * w[0] * sq
                  - 0.5 * w[1] * (kld + BS * Z)
                  + (w[2] / B) * ce)
    return np.float32(total)
